# revision 27
# baseline (speedup 1.0000x reference)
"""Longformer attention Trainium2 kernel (8 NeuronCores, SPMD).

Sharding: data-parallel over batch (cores 0-3 -> batch 0, 4-7 -> batch 1),
head-parallel within a batch group (4 heads = 256 channels per core).
Each core: QKV projection (fp8 DoubleRow matmuls) for its head slice,
banded+global attention (bf16), out-projection partial (fp8 DoubleRow);
host sums the 4 bf16 partials per batch and adds bias terms (bo and the
fold of bv through Wo).  The K bias is dropped on-device: it shifts every
score in a softmax row by the same constant, so softmax is invariant.
"""

import numpy as np
import ml_dtypes

import concourse.bacc as bacc
import concourse.mybir as mybir
from concourse.tile import TileContext
from concourse.bass_utils import run_bass_kernel_spmd

S = 2048          # sequence length
D = 1024          # model dim
NH = 16           # total heads
DH = 64           # head dim
HPC = 4           # heads per core
CPB = 4           # cores per batch
WIN = 256         # attention window (2 blocks of 128)
NB = S // 128     # 16 query/key blocks
BF16 = mybir.dt.bfloat16
F32 = mybir.dt.float32
F8 = mybir.dt.float8e4
I16 = mybir.dt.int16
DR = mybir.MatmulPerfMode.DoubleRow
SCH_A = 128.0 / np.log(2.0)      # Schraudolph exp: bf16 = bitcast(i16(A*x + B))
SCH_B = 127.0 * 128.0 - 6.0

_CACHE = {}


def _band(qb):
    return list(range(max(0, qb - 2), min(NB - 1, qb + 2) + 1))


def _mask_id(qb, kb):
    # 0:M1 lower edge, 1:M1g (+global key row), 2:M2 upper edge, 3:M2g (+global query col)
    if kb == qb - 2:
        return 1 if kb == 0 else 0
    if kb == qb + 2:
        return 3 if qb == 0 else 2
    return None


def build_masks():
    """Additive masks (applied to scores in PSUM via identity matmul) plus
    the identity matrix itself in slot 4."""
    ki = np.arange(128)[:, None]
    qi = np.arange(128)[None, :]
    m1 = (qi <= ki).astype(np.float32)          # kb == qb-2 : valid iff qi <= ki
    m2 = (ki <= qi).astype(np.float32)          # kb == qb+2 : valid iff ki <= qi
    m1g = m1.copy(); m1g[0, :] = 1.0            # global key k=0 row
    m2g = m2.copy(); m2g[:, 0] = 1.0            # global query q=0 col
    mm = np.stack([m1, m1g, m2, m2g])
    add = -30.0 * (1.0 - mm)
    return np.concatenate([add, np.eye(128, dtype=np.float32)[None]],
                          axis=0).astype(ml_dtypes.bfloat16)


def build_program():
    nc = bacc.Bacc("TRN2", target_bir_lowering=False, debug=False, num_devices=8)

    x8d = nc.dram_tensor("x8", [128, 8, S], F8, kind="ExternalInput").ap()
    wq = nc.dram_tensor("wq", [128, 8, 256], F8, kind="ExternalInput").ap()
    wk = nc.dram_tensor("wk", [128, 8, 256], F8, kind="ExternalInput").ap()
    wv = nc.dram_tensor("wv", [128, 8, 256], F8, kind="ExternalInput").ap()
    wo = nc.dram_tensor("wo", [128, 2, D], F8, kind="ExternalInput").ap()
    bqd = nc.dram_tensor("bq", [2, 128, 1], F32, kind="ExternalInput").ap()
    maskd = nc.dram_tensor("masks", [5, 128, 128], BF16, kind="ExternalInput").ap()
    y = nc.dram_tensor("y", [S, D], BF16, kind="ExternalOutput").ap()

    with TileContext(nc) as tc:
        import contextlib
        with contextlib.ExitStack() as ctx, \
                nc.allow_low_precision(reason="fp8/bf16 attention interior by design"):
            sbw = ctx.enter_context(tc.tile_pool(name="sbw", bufs=1))
            sbx = ctx.enter_context(tc.tile_pool(name="sbx", bufs=1))
            sbqk = ctx.enter_context(tc.tile_pool(name="sbqk", bufs=1))
            sbes = ctx.enter_context(tc.tile_pool(name="sbes", bufs=16))
            sbsm = ctx.enter_context(tc.tile_pool(name="sbsm", bufs=4))
            sbbc = ctx.enter_context(tc.tile_pool(name="sbbc", bufs=4))
            sbrc = ctx.enter_context(tc.tile_pool(name="sbrc", bufs=3))
            psA = ctx.enter_context(tc.tile_pool(name="psA", bufs=2, space="PSUM"))
            psS = ctx.enter_context(tc.tile_pool(name="psS", bufs=2, space="PSUM"))
            psPV = ctx.enter_context(tc.tile_pool(name="psPV", bufs=2, space="PSUM"))

            # ---- load inputs ----
            wqt = sbw.tile([128, 8, 256], F8, tag="wq")
            nc.sync.dma_start(out=wqt[:], in_=wq[:, :, :])
            x8 = sbx.tile([128, 8, S], F8, tag="x8")
            nc.sync.dma_start(out=x8[:, :, 0:512], in_=x8d[:, :, 0:512])
            wkt = sbw.tile([128, 8, 256], F8, tag="wk")
            nc.gpsimd.dma_start(out=wkt[:], in_=wk[:, :, :])
            nc.gpsimd.dma_start(out=x8[:, :, 512:1024], in_=x8d[:, :, 512:1024])
            bqt = []
            for cc in range(2):
                tq = sbw.tile([128, 1], F32, tag=f"bq{cc}")
                nc.sync.dma_start(out=tq[:], in_=bqd[cc, :, :])
                bqt.append(tq)
            ones1 = sbw.tile([1, 128], BF16, tag="ones1")
            nc.vector.memset(ones1[:], 1.0)
            mt = []
            for i in range(5):
                t = sbw.tile([128, 128], BF16, tag=f"mask{i}")
                nc.sync.dma_start(out=t[:], in_=maskd[i, :, :])
                mt.append(t)
            ident = mt[4]
            wvt = sbw.tile([128, 8, 256], F8, tag="wv")
            nc.gpsimd.dma_start(out=wvt[:], in_=wv[:, :, :])
            nc.sync.dma_start(out=x8[:, :, 1024:1536], in_=x8d[:, :, 1024:1536])
            nc.gpsimd.dma_start(out=x8[:, :, 1536:2048], in_=x8d[:, :, 1536:2048])
            wot = sbw.tile([128, 2, D], F8, tag="wo")
            nc.sync.dma_start(out=wot[:], in_=wo[:, :, :])

            # ---- persistent intermediates ----
            QT = [sbqk.tile([128, S], BF16, tag=f"QT{c}", name=f"QT{c}") for c in range(2)]
            KT = [sbqk.tile([128, S], BF16, tag=f"KT{c}", name=f"KT{c}") for c in range(2)]
            Vo = sbqk.tile([128, NB, HPC * 65], F8, tag="Vo", name="Vo")
            AO8 = sbqk.tile([128, 2, S], F8, tag="AO8", name="AO8")

            # ---- phase A: projections (fp8 DoubleRow), span-by-span ----
            def emit_qkt_span(ts):
                sp = slice(ts * 512, (ts + 1) * 512)
                for cc in range(2):
                    pq = psA.tile([128, 512], F32, tag="psA", name="pq")
                    for g in range(4):
                        nc.tensor.matmul(pq[:], wqt[:, 2 * g:2 * g + 2, cc * 128:(cc + 1) * 128],
                                         x8[:, 2 * g:2 * g + 2, sp],
                                         start=(g == 0), stop=(g == 3), perf_mode=DR)
                    # Q' = (x Wq + bq) / 8 : scale folded in, bias pre-scaled on host
                    nc.vector.tensor_scalar(QT[cc][:, sp], pq[:], 0.125, bqt[cc][:],
                                            mybir.AluOpType.mult, mybir.AluOpType.add)
                    pk = psA.tile([128, 512], F32, tag="psA", name="pk")
                    for g in range(4):
                        nc.tensor.matmul(pk[:], wkt[:, 2 * g:2 * g + 2, cc * 128:(cc + 1) * 128],
                                         x8[:, 2 * g:2 * g + 2, sp],
                                         start=(g == 0), stop=(g == 3), perf_mode=DR)
                    nc.scalar.activation(KT[cc][:, sp], pk[:],
                                         mybir.ActivationFunctionType.Copy)

            def emit_v(tb):
                pv = psA.tile([128, 256], F32, tag="psA", name="pv")
                for g in range(4):
                    nc.tensor.matmul(pv[:], x8[:, 2 * g:2 * g + 2, tb * 128:(tb + 1) * 128],
                                     wvt[:, 2 * g:2 * g + 2, :],
                                     start=(g == 0), stop=(g == 3), perf_mode=DR)
                # scatter heads into [h*65 : h*65+64]; col h*65+64 gets ones
                vtb = Vo[:, tb, 0:260].rearrange("p (h c) -> p h c", h=4)
                inap = pv[:].rearrange("p (h c) -> p h c", h=4)
                nc.vector.tensor_copy(vtb[:, :, 0:64], inap)
                nc.vector.memset(vtb[:, :, 64:65], 1.0)

            # ---- global key (k=0) score rows, batched 4 qb per exp ----
            # esgt[h][g] covers qb 4g..4g+3 as [1, 512]; only slices for qb>=3 used
            esgt = [[None] * 4 for _ in range(HPC)]
            def emit_esg(g):
                for h in range(HPC):
                    hp, r0 = h // 2, (h % 2) * 64
                    psg = psA.tile([128, 512], F32, tag="psA", name="psg")
                    for j in range(4):
                        qb = 4 * g + j
                        if qb < 3:
                            continue
                        nc.tensor.matmul(psg[0:1, j * 128:(j + 1) * 128],
                                         KT[hp][r0:r0 + 64, 0:1],
                                         QT[hp][r0:r0 + 64, qb * 128:(qb + 1) * 128],
                                         start=True, stop=True)
                    eg = sbsm.tile([1, 512], F8, tag=f"esg{h}_{g}", name="eg")
                    lo = 3 if g == 0 else 0
                    nc.scalar.activation(eg[0:1, lo * 128:512], psg[0:1, lo * 128:512],
                                         mybir.ActivationFunctionType.Exp)
                    esgt[h][g] = eg

            emit_qkt_span(0)
            emit_esg(0)
            emit_qkt_span(1)
            emit_esg(1)

            # ---- banded attention; V tiles emitted just-in-time so exp work
            # ---- starts early; pair 0 (which needs all V for the global row)
            # ---- runs after pair 4
            pair_order = [1, 2, 3, 4, 0, 5, 6, 7]
            v_before = {1: range(0, 6), 2: range(6, 8), 3: range(8, 10),
                        4: range(10, 12), 0: range(12, 16)}
            for pair in pair_order:
                if pair == 2:
                    emit_qkt_span(2)
                    emit_esg(2)
                    emit_qkt_span(3)
                    emit_esg(3)
                for tb in v_before.get(pair, ()):
                    emit_v(tb)
                qb0 = pair * 2
                rec4 = sbrc.tile([1, 1024], BF16, tag="rec4", name="rec4")
                ao_tmps = {}
                for hp in range(2):
                    # both heads of the pair together: adjacent S^T matmuls hit
                    # different PE row-groups (partitions 0-63 vs 64-127) and
                    # overlap in the array
                    ppvs, jobs = {}, {0: [], 1: []}
                    for h2 in range(2):
                        ppvs[h2] = psPV.tile([65, 256], F32, tag="ppv", name="ppv")
                    for sub in range(2):
                        qb = qb0 + sub
                        qs = slice(qb * 128, (qb + 1) * 128)
                        kbs = _band(qb)
                        w = len(kbs) * 128
                        pss, ess = {}, {}
                        for h2 in range(2):
                            pss[h2] = psS.tile([128, 640], F32, tag="psS", name="ps")
                        for i, kb in enumerate(kbs):
                            mid = _mask_id(qb, kb)
                            for h2 in range(2):
                                r0 = h2 * 64
                                sl = slice(i * 128, (i + 1) * 128)
                                nc.tensor.matmul(pss[h2][:, sl],
                                                 KT[hp][r0:r0 + 64, kb * 128:(kb + 1) * 128],
                                                 QT[hp][r0:r0 + 64, qs],
                                                 start=True, stop=(mid is None))
                                if mid is not None:
                                    # additive mask: out += I.T @ mask = mask
                                    nc.tensor.matmul(pss[h2][:, sl], ident[:],
                                                     mt[mid][:], start=False, stop=True)
                        # exp: split between Act (fp8 out) and DVE (Schraudolph
                        # int16-bitcast bf16); DVE-made tiles feed single
                        # (non-DoubleRow) PV matmuls
                        dve_exp = set()
                        for h2 in range(2):
                            if (sub, h2) in dve_exp:
                                esi = sbes.tile([128, 640], I16, tag="es", name="esi")
                                nc.vector.tensor_scalar(esi[:, 0:w], pss[h2][:, 0:w],
                                                        SCH_A, SCH_B,
                                                        mybir.AluOpType.mult,
                                                        mybir.AluOpType.add)
                                ess[h2] = (esi[:].bitcast(BF16), False)
                            else:
                                es = sbes.tile([128, 640], F8, tag="es", name="es")
                                nc.scalar.activation(es[:, 0:w], pss[h2][:, 0:w],
                                                     mybir.ActivationFunctionType.Exp)
                                ess[h2] = (es[:], True)
                        for h2 in range(2):
                            h = hp * 2 + h2
                            r0 = h2 * 64
                            hs = slice(h * 65, h * 65 + 65)
                            ov = ppvs[h2][:, sub * 128:(sub + 1) * 128]
                            esap, is_f8 = ess[h2]
                            nkb = len(kbs)
                            i = 0
                            while i < nkb:   # DoubleRow over adjacent kb pairs
                                if i + 1 < nkb and is_f8:
                                    rh = esap[:, i * 128:(i + 2) * 128].rearrange(
                                        "p (t q) -> p t q", t=2)
                                    jobs[h2].append((Vo[:, kbs[i]:kbs[i] + 2, hs],
                                                     rh, ov, i == 0, sub, DR))
                                    i += 2
                                else:
                                    jobs[h2].append((Vo[:, kbs[i], hs],
                                                     esap[:, i * 128:(i + 1) * 128],
                                                     ov, i == 0, sub, None))
                                    i += 1
                            if qb >= 3:  # global key k=0 column
                                eg = esgt[h][qb // 4]
                                co = (qb % 4) * 128
                                jobs[h2].append((Vo[0:1, 0, hs], eg[0:1, co:co + 128],
                                                 ov, False, sub, None))
                            if qb == 0:  # global query q=0 vs far keys
                                ps0 = psA.tile([128, 512], F32, tag="psA", name="ps0")
                                for i, kb in enumerate(range(3, NB)):
                                    nc.tensor.matmul(
                                        ps0[:, i:i + 1],
                                        KT[hp][r0:r0 + 64, kb * 128:(kb + 1) * 128],
                                        QT[hp][r0:r0 + 64, 0:1], start=True, stop=True)
                                es0 = sbsm.tile([128, 13], F8, tag="es0", name="es0")
                                nc.scalar.activation(es0[:], ps0[:, 0:13],
                                                     mybir.ActivationFunctionType.Exp)
                                for i, kb in enumerate(range(3, NB)):
                                    jobs[h2].append((Vo[:, kb, hs], es0[:, i:i + 1],
                                                     ppvs[h2][:, 0:1], False, sub, None))
                    for h2 in range(2):
                        h = hp * 2 + h2
                        pv_jobs = jobs[h2]
                        last_of_sub = {s: max(i for i, j in enumerate(pv_jobs)
                                              if j[4] == s) for s in (0, 1)}
                        for i_mm, (lh, rh, ov, first, sub, pm) in enumerate(pv_jobs):
                            nc.tensor.matmul(ov, lh, rh, start=first,
                                             stop=(i_mm == last_of_sub[sub]),
                                             perf_mode=pm)
                        nc.vector.reciprocal(rec4[0:1, h * 256:(h + 1) * 256],
                                             ppvs[h2][64:65, :])
                        ao_tmp = sbbc.tile([64, 256], BF16, tag="aotmp", name="ao_tmp")
                        nc.vector.tensor_copy(ao_tmp[:], ppvs[h2][0:64, :])
                        ao_tmps[h] = ao_tmp

                # ---- normalize into fp8 AO8 on Pool: broadcast the pair's
                # ---- reciprocals then per-head multiply (all SBUF)
                psp = slice(qb0 * 128, (qb0 + 2) * 128)
                pbS = sbbc.tile([64, 1024], BF16, tag="pbS", name="pbS")
                nc.gpsimd.partition_broadcast(pbS[:], rec4[0:1, :])
                for h in range(HPC):
                    r0, cc = (h % 2) * 64, h // 2
                    nc.gpsimd.tensor_mul(AO8[r0:r0 + 64, cc, psp], ao_tmps[h][:],
                                         pbS[:, h * 256:(h + 1) * 256])
                for qb2 in (qb0, qb0 + 1):
                    q2 = slice(qb2 * 128, (qb2 + 1) * 128)
                    for eh in range(2):
                        po = psA.tile([128, 512], F32, tag="psA", name="po")
                        nc.tensor.matmul(po[:], AO8[:, :, q2],
                                         wot[:, :, eh * 512:(eh + 1) * 512],
                                         start=True, stop=True, perf_mode=DR)
                        ys = sbbc.tile([128, 512], BF16, tag="ystage", name="ys")
                        if eh == 0:
                            nc.scalar.activation(ys[:], po[:],
                                                 mybir.ActivationFunctionType.Copy)
                        else:
                            nc.vector.tensor_copy(ys[:], po[:])
                        eng = nc.sync if (qb2 + eh) % 2 else nc.gpsimd
                        eng.dma_start(out=y[q2, eh * 512:(eh + 1) * 512], in_=ys[:])

    nc.compile()
    return nc


def kernel(x, Wq, bq, Wk, bk, Wv, bv, Wo, bo):
    x = np.asarray(x); Wq = np.asarray(Wq); bq = np.asarray(bq)
    Wk = np.asarray(Wk); bk = np.asarray(bk); Wv = np.asarray(Wv)
    bv = np.asarray(bv); Wo = np.asarray(Wo); bo = np.asarray(bo)
    if "nc" not in _CACHE:
        _CACHE["nc"] = build_program()
    nc = _CACHE["nc"]

    B = x.shape[0]
    masks = build_masks()
    f8 = ml_dtypes.float8_e4m3
    in_maps = []
    for c in range(8):
        b = c // CPB
        h0 = (c % CPB) * HPC * DH          # channel offset of this core's heads
        sl = slice(h0, h0 + HPC * DH)
        # x8[p, t, n] = x[b][n, 128t+p]
        x8 = np.ascontiguousarray(
            x[b].T.reshape(8, 128, S).transpose(1, 0, 2)).astype(f8)
        def wprep(W):   # [p, t, m] = W[128t+p, sl][.., m]
            return np.ascontiguousarray(
                W[:, sl].reshape(8, 128, 256).transpose(1, 0, 2)).astype(f8)
        wo8 = np.ascontiguousarray(
            Wo[sl, :].reshape(2, 128, D).transpose(1, 0, 2)).astype(f8)
        in_maps.append({
            "x8": x8,
            "wq": wprep(Wq),
            "wk": wprep(Wk),
            "wv": wprep(Wv),
            "wo": wo8,
            "bq": (bq[sl] * 0.125).reshape(2, 128, 1).astype(np.float32),
            "masks": masks,
        })
    res = run_bass_kernel_spmd(nc, in_maps, list(range(8)))
    out = np.zeros((B, S, D), dtype=np.float32)
    for c in range(8):
        out[c // CPB] += res.results[c]["y"].astype(np.float32)
    out += (bv @ Wo + bo)[None, None, :]
    return out
